# revision 19
# baseline (speedup 1.0000x reference)
"""KL-divergence kNN retrieval kernel for 8 TRN2 NeuronCores.

reference math:
  kl[b,k]  = mean_d a[k,d]*(log a[k,d] - log q[b,d])
           = a_ent[k] - (A @ log(Q).T)[k,b] / D
  top-16 smallest kl per query -> label vote -> argmax.

device (per core, anchors sharded K/8=1024):
  score[k,b] = sum_d a[k,d]*ln q[b,d] - sum_d a[k,d]*ln a[k,d]   (= -D*kl)
  larger score = nearer neighbor.  GEMM runs on TensorE in float32r
  (fp22 multiply, fp32 accumulate) at full PE rate; operands are
  PE-transposed into [d, *] layout; entropy term a*ln(a) summed in fp32
  on VectorE.  Host merges the 8 slabs, global top-16, label vote.
"""

import sys

import numpy as np

sys.path.insert(0, "/opt/trn_rl_repo")

from contextlib import ExitStack

import concourse.tile as tile
from concourse import bacc, mybir
from concourse.bass_utils import run_bass_kernel_spmd
from concourse.masks import make_identity

B, K, D = 512, 8192, 50257
NCORES = 8
KS = K // NCORES  # anchors per core
KNN = 16
N_CLASS = 4
P = 128
SUPER = 1024  # d-columns per superchunk

F32 = mybir.dt.float32
F32R = mybir.dt.float32r
LN = mybir.ActivationFunctionType.Ln
MULT = mybir.AluOpType.mult
ADD = mybir.AluOpType.add


def build_kernel(b=B, ks=KS, d=D, use_f32r=True, tr_f32r=False):
    nc = bacc.Bacc(None, target_bir_lowering=False, debug=False)
    q_ext = nc.declare_dram_parameter("query", [b, d], F32, isOutput=False)
    a_ext = nc.declare_dram_parameter("anchor", [ks, d], F32, isOutput=False)
    out_ext = nc.declare_dram_parameter("out", [ks, b], F32, isOutput=True)

    MB = ks // P  # anchor blocks per core
    QB = b // P  # query blocks
    n_super = (d + SUPER - 1) // SUPER

    MMDT = F32R if use_f32r else F32
    TRDT = F32R if (use_f32r and tr_f32r) else F32

    def as_f32(ap):
        return ap.bitcast(F32) if TRDT == F32R else ap

    with tile.TileContext(nc) as tc, ExitStack() as ctx:
        const_pool = ctx.enter_context(tc.tile_pool(name="const", bufs=1))
        identf = const_pool.tile([P, P], F32)
        make_identity(nc, identf[:])
        if TRDT == F32R:
            # f32r identity: transposes at 1.5 cycles/row instead of fp32's 4
            ident = const_pool.tile([P, P], F32R)
            nc.vector.tensor_copy(ident[:], identf[:])
        else:
            ident = identf
        # per-partition entropy accumulators, one column per anchor block
        aent = const_pool.tile([P, MB], F32)
        # per-superchunk entropy partial sums (reduced at the end)
        aent_parts = [
            const_pool.tile([P, n_super], F32, tag=f"aentp{m}", name=f"aentp{m}")
            for m in range(MB)
        ]
        # persistent score accumulators in SBUF
        acc_pool = ctx.enter_context(tc.tile_pool(name="acc", bufs=1))
        acc = [acc_pool.tile([P, b], F32, tag=f"acc{m}", name=f"acc{m}") for m in range(MB)]

        anat_pool = ctx.enter_context(tc.tile_pool(name="anat", bufs=MB + 2))
        qnat_pool = ctx.enter_context(tc.tile_pool(name="qnat", bufs=QB + 2))
        lna_pool = ctx.enter_context(tc.tile_pool(name="lna", bufs=3))
        at_pool = ctx.enter_context(tc.tile_pool(name="at", bufs=10))
        lnqt_pool = ctx.enter_context(tc.tile_pool(name="lnqt", bufs=10))
        psA = ctx.enter_context(tc.tile_pool(name="psA", bufs=2, space="PSUM"))
        psQ = ctx.enter_context(tc.tile_pool(name="psQ", bufs=2, space="PSUM"))
        psM = ctx.enter_context(tc.tile_pool(name="psM", bufs=3, space="PSUM"))

        for s in range(n_super):
            d0 = s * SUPER
            w = min(SUPER, d - d0)
            nch = (w + P - 1) // P

            # ---- load naturals, entropy term ----
            a_nat = []
            for m in range(MB):
                t = anat_pool.tile([P, w], TRDT, tag="anat", name="anat")
                nc.sync.dma_start(
                    t[:], a_ext[P * m : P * (m + 1), d0 : d0 + w].bitcast(TRDT)
                )
                a_nat.append(t)
                ln_a = lna_pool.tile([P, w], F32, tag="lna", name="lna")
                nc.scalar.activation(ln_a[:], as_f32(t[:]), LN)
                nc.vector.tensor_tensor(ln_a[:], as_f32(t[:]), ln_a[:], MULT)
                nc.vector.tensor_reduce(
                    aent_parts[m][:, s : s + 1], ln_a[:], mybir.AxisListType.X, ADD
                )
            q_nat = []
            for qb in range(QB):
                t = qnat_pool.tile([P, w], TRDT, tag="qnat", name="qnat")
                nc.sync.dma_start(
                    t[:], q_ext[P * qb : P * (qb + 1), d0 : d0 + w].bitcast(TRDT)
                )
                q_nat.append(t)

            # ---- transposes ----
            at_sb = []
            lnq_t = []
            for j in range(nch):
                ck = min(P, w - j * P)
                a_t = at_pool.tile([P, MB * P], MMDT, tag="at", name="at")
                for h in range((MB + 3) // 4):
                    pa = psA.tile([P, 512], TRDT, tag="psA", name="psA")
                    mlo, mhi = 4 * h, min(4 * h + 4, MB)
                    for m in range(mlo, mhi):
                        nc.tensor.transpose(
                            pa[:ck, (m - mlo) * P : (m - mlo + 1) * P],
                            a_nat[m][:, j * P : j * P + ck],
                            ident[:],
                        )
                    nc.any.tensor_copy(
                        out=a_t[:ck, mlo * P : mhi * P],
                        in_=pa[:ck, : (mhi - mlo) * P],
                    )
                at_sb.append(a_t)

                pq = psQ.tile([P, QB * P], TRDT, tag="psQ", name="psQ")
                for qb in range(QB):
                    nc.tensor.transpose(
                        pq[:ck, qb * P : (qb + 1) * P],
                        q_nat[qb][:, j * P : j * P + ck],
                        ident[:],
                    )
                lq = lnqt_pool.tile([P, QB * P], MMDT, tag="lnqt", name="lnqt")
                nc.scalar.activation(lq[:ck, :], as_f32(pq[:ck, :]), LN)
                lnq_t.append(lq)

            # ---- GEMM: for each anchor block accumulate over this superchunk ----
            for m in range(MB):
                pm = psM.tile([P, b], F32, tag="psM", name="psM")
                for j in range(nch):
                    ck = min(P, w - j * P)
                    nc.tensor.matmul(
                        pm[:, :],
                        at_sb[j][:ck, m * P : (m + 1) * P],
                        lnq_t[j][:ck, :],
                        start=(j == 0),
                        stop=(j == nch - 1),
                    )
                if s == 0:
                    nc.any.tensor_copy(out=acc[m][:], in_=pm[:])
                else:
                    nc.any.tensor_tensor(acc[m][:], acc[m][:], pm[:], ADD)

        # ---- epilogue: score = acc - aent, DMA out ----
        for m in range(MB):
            nc.vector.tensor_reduce(
                aent[:, m : m + 1], aent_parts[m][:], mybir.AxisListType.X, ADD
            )
            nc.vector.tensor_scalar_sub(acc[m][:], acc[m][:], aent[:, m : m + 1])
            nc.sync.dma_start(out_ext[P * m : P * (m + 1), :], acc[m][:])

    nc.compile()
    return nc


_CACHE = {}


def _get_kernel(b, ks, d):
    key = (b, ks, d)
    if key not in _CACHE:
        _CACHE[key] = build_kernel(b=b, ks=ks, d=d)
    return _CACHE[key]


def run_cores(query, anchor, trace=False):
    """query [B,D] f32, anchor [K,D] f32 -> score slab [K,B] f32, results obj."""
    b, d = query.shape
    k = anchor.shape[0]
    ks = k // NCORES
    nc = _get_kernel(b, ks, d)
    query = np.ascontiguousarray(query, dtype=np.float32)
    anchor = np.ascontiguousarray(anchor, dtype=np.float32)
    in_maps = [
        {"query": query, "anchor": anchor[c * ks : (c + 1) * ks]}
        for c in range(NCORES)
    ]
    res = run_bass_kernel_spmd(nc, in_maps, core_ids=list(range(NCORES)), trace=trace)
    slab = np.concatenate([res.results[c]["out"] for c in range(NCORES)], axis=0)
    return slab, res


def finish_host(score_slab, queue_label):
    """score_slab [K,B] (larger=nearer) -> voted labels [B] int32."""
    St = score_slab.T  # [B, K]
    k = St.shape[1]
    ncand = min(64, k)
    if ncand < k:
        part = np.argpartition(-St, ncand - 1, axis=1)[:, :ncand]
    else:
        part = np.broadcast_to(np.arange(k), St.shape).copy()
    vals = np.take_along_axis(St, part, axis=1)
    # sort candidates: descending score, ties -> lower anchor index (top_k semantics)
    order = np.lexsort((part, -vals), axis=1)[:, :KNN]
    idx = np.take_along_axis(part, order, axis=1)
    knn_labels = np.asarray(queue_label)[idx]  # [B, KNN]
    counts = np.zeros((St.shape[0], N_CLASS), dtype=np.int64)
    for c in range(N_CLASS):
        counts[:, c] = (knn_labels == c).sum(axis=1)
    return np.argmax(counts, axis=1).astype(np.int32)


def kernel(query, queue_anchor, queue_label):
    slab, _ = run_cores(query, queue_anchor, trace=False)
    return finish_host(slab, queue_label)


if __name__ == "__main__":
    # small smoke test: D small, KS small
    import time

    rng = np.random.default_rng(0)
    d_s, ks_s, b_s = 2048 + 81, 256, 512
    q = rng.uniform(1e-6, 1.0, (b_s, d_s)).astype(np.float32)
    a = rng.uniform(1e-6, 1.0, (ks_s * NCORES, d_s)).astype(np.float32)
    t0 = time.time()
    nc = _get_kernel(b_s, ks_s, d_s)
    print(f"build+compile: {time.time() - t0:.1f}s")
    slab, res = run_cores(q, a)
    # numpy oracle in fp64
    a64 = a.astype(np.float64)
    lnq = np.log(q.astype(np.float64))
    want = a64 @ lnq.T - (a64 * np.log(a64)).sum(axis=1, keepdims=True)
    err = np.abs(slab - want)
    denom = np.maximum(np.abs(want), 1e-30)
    print(f"max abs err {err.max():.4e}  max rel {np.max(err / denom):.4e}")
    print(f"sample got {slab[:2, :3]}\nwant {want[:2, :3]}")


# revision 23
# speedup vs baseline: 1.0847x; 1.0847x over previous
"""KL-divergence kNN retrieval kernel for 8 TRN2 NeuronCores.

reference math:
  kl[b,k]  = mean_d a[k,d]*(log a[k,d] - log q[b,d])
           = a_ent[k] - (A @ log(Q).T)[k,b] / D
  top-16 smallest kl per query -> label vote -> argmax.

device (per core, anchors sharded K/8=1024):
  score[k,b] = sum_d a[k,d]*ln q[b,d] - sum_d a[k,d]*ln a[k,d]   (= -D*kl)
  larger score = nearer neighbor.  GEMM runs on TensorE in float32r
  (fp22 multiply, fp32 accumulate) at full PE rate; operands are
  PE-transposed into [d, *] layout; entropy term a*ln(a) summed in fp32
  on VectorE.  Host merges the 8 slabs, global top-16, label vote.
"""

import sys

import numpy as np

sys.path.insert(0, "/opt/trn_rl_repo")

from contextlib import ExitStack

import concourse.tile as tile
from concourse import bacc, mybir
from concourse.bass_utils import run_bass_kernel_spmd
from concourse.masks import make_identity

B, K, D = 512, 8192, 50257
NCORES = 8
KS = K // NCORES  # anchors per core
KNN = 16
N_CLASS = 4
P = 128
SUPER = 1024  # d-columns per superchunk

F32 = mybir.dt.float32
F32R = mybir.dt.float32r
LN = mybir.ActivationFunctionType.Ln
MULT = mybir.AluOpType.mult
ADD = mybir.AluOpType.add


def build_kernel(b=B, ks=KS, d=D, use_f32r=True, tr_f32r=False):
    nc = bacc.Bacc(None, target_bir_lowering=False, debug=False)
    q_ext = nc.declare_dram_parameter("query", [b, d], F32, isOutput=False)
    a_ext = nc.declare_dram_parameter("anchor", [ks, d], F32, isOutput=False)
    out_ext = nc.declare_dram_parameter("out", [ks, b], F32, isOutput=True)

    MB = ks // P  # anchor blocks per core
    QB = b // P  # query blocks
    n_super = (d + SUPER - 1) // SUPER

    MMDT = F32R if use_f32r else F32
    TRDT = F32R if (use_f32r and tr_f32r) else F32

    def as_f32(ap):
        return ap.bitcast(F32) if TRDT == F32R else ap

    with tile.TileContext(nc) as tc, ExitStack() as ctx:
        const_pool = ctx.enter_context(tc.tile_pool(name="const", bufs=1))
        identf = const_pool.tile([P, P], F32)
        make_identity(nc, identf[:])
        if TRDT == F32R:
            # f32r identity: transposes at 1.5 cycles/row instead of fp32's 4
            ident = const_pool.tile([P, P], F32R)
            nc.vector.tensor_copy(ident[:], identf[:])
        else:
            ident = identf
        # per-partition entropy accumulators, one column per anchor block
        aent = const_pool.tile([P, MB], F32)
        # per-superchunk entropy partial sums (reduced at the end)
        aent_parts = [
            const_pool.tile([P, n_super], F32, tag=f"aentp{m}", name=f"aentp{m}")
            for m in range(MB)
        ]
        # persistent score accumulators in SBUF
        acc_pool = ctx.enter_context(tc.tile_pool(name="acc", bufs=1))
        acc = [acc_pool.tile([P, b], F32, tag=f"acc{m}", name=f"acc{m}") for m in range(MB)]

        anat_pool = ctx.enter_context(tc.tile_pool(name="anat", bufs=2 * MB))
        qnat_pool = ctx.enter_context(tc.tile_pool(name="qnat", bufs=2 * QB))
        lna_pool = ctx.enter_context(tc.tile_pool(name="lna", bufs=3))
        at_pool = ctx.enter_context(tc.tile_pool(name="at", bufs=9))
        lnqt_pool = ctx.enter_context(tc.tile_pool(name="lnqt", bufs=9))
        psA = ctx.enter_context(tc.tile_pool(name="psA", bufs=2, space="PSUM"))
        psQ = ctx.enter_context(tc.tile_pool(name="psQ", bufs=2, space="PSUM"))
        psM = ctx.enter_context(tc.tile_pool(name="psM", bufs=4, space="PSUM"))

        for s in range(n_super):
            d0 = s * SUPER
            w = min(SUPER, d - d0)
            nch = (w + P - 1) // P

            # ---- load naturals, entropy term ----
            a_nat = []
            for m in range(MB):
                t = anat_pool.tile([P, w], TRDT, tag="anat", name="anat")
                nc.sync.dma_start(
                    t[:], a_ext[P * m : P * (m + 1), d0 : d0 + w].bitcast(TRDT)
                )
                a_nat.append(t)
                ln_a = lna_pool.tile([P, w], F32, tag="lna", name="lna")
                nc.scalar.activation(ln_a[:], as_f32(t[:]), LN)
                nc.vector.tensor_tensor(ln_a[:], as_f32(t[:]), ln_a[:], MULT)
                nc.vector.tensor_reduce(
                    aent_parts[m][:, s : s + 1], ln_a[:], mybir.AxisListType.X, ADD
                )
            q_nat = []
            for qb in range(QB):
                t = qnat_pool.tile([P, w], TRDT, tag="qnat", name="qnat")
                nc.sync.dma_start(
                    t[:], q_ext[P * qb : P * (qb + 1), d0 : d0 + w].bitcast(TRDT)
                )
                q_nat.append(t)

            # ---- transposes ----
            at_sb = []
            lnq_t = []
            for j in range(nch):
                ck = min(P, w - j * P)
                a_t = at_pool.tile([P, MB * P], MMDT, tag="at", name="at")
                for h in range((MB + 3) // 4):
                    pa = psA.tile([P, 512], TRDT, tag="psA", name="psA")
                    mlo, mhi = 4 * h, min(4 * h + 4, MB)
                    for m in range(mlo, mhi):
                        nc.tensor.transpose(
                            pa[:ck, (m - mlo) * P : (m - mlo + 1) * P],
                            a_nat[m][:, j * P : j * P + ck],
                            ident[:],
                        )
                    nc.any.tensor_copy(
                        out=a_t[:ck, mlo * P : mhi * P],
                        in_=pa[:ck, : (mhi - mlo) * P],
                    )
                at_sb.append(a_t)

                pq = psQ.tile([P, QB * P], TRDT, tag="psQ", name="psQ")
                for qb in range(QB):
                    nc.tensor.transpose(
                        pq[:ck, qb * P : (qb + 1) * P],
                        q_nat[qb][:, j * P : j * P + ck],
                        ident[:],
                    )
                lq = lnqt_pool.tile([P, QB * P], MMDT, tag="lnqt", name="lnqt")
                nc.scalar.activation(lq[:ck, :], as_f32(pq[:ck, :]), LN)
                lnq_t.append(lq)

            # ---- GEMM: for each anchor block accumulate over this superchunk ----
            for m in range(MB):
                pm = psM.tile([P, b], F32, tag="psM", name="psM")
                for j in range(nch):
                    ck = min(P, w - j * P)
                    nc.tensor.matmul(
                        pm[:, :],
                        at_sb[j][:ck, m * P : (m + 1) * P],
                        lnq_t[j][:ck, :],
                        start=(j == 0),
                        stop=(j == nch - 1),
                    )
                if s == 0:
                    nc.any.tensor_copy(out=acc[m][:], in_=pm[:])
                else:
                    nc.any.tensor_tensor(acc[m][:], acc[m][:], pm[:], ADD)
                if s == n_super - 1:
                    # fused epilogue: finish block m as soon as its last add lands
                    nc.vector.tensor_reduce(
                        aent[:, m : m + 1], aent_parts[m][:], mybir.AxisListType.X, ADD
                    )
                    nc.vector.tensor_scalar_sub(
                        acc[m][:], acc[m][:], aent[:, m : m + 1]
                    )
                    nc.sync.dma_start(out_ext[P * m : P * (m + 1), :], acc[m][:])

    nc.compile()
    return nc


_CACHE = {}


def _get_kernel(b, ks, d):
    key = (b, ks, d)
    if key not in _CACHE:
        _CACHE[key] = build_kernel(b=b, ks=ks, d=d)
    return _CACHE[key]


def run_cores(query, anchor, trace=False):
    """query [B,D] f32, anchor [K,D] f32 -> score slab [K,B] f32, results obj."""
    b, d = query.shape
    k = anchor.shape[0]
    ks = k // NCORES
    nc = _get_kernel(b, ks, d)
    query = np.ascontiguousarray(query, dtype=np.float32)
    anchor = np.ascontiguousarray(anchor, dtype=np.float32)
    in_maps = [
        {"query": query, "anchor": anchor[c * ks : (c + 1) * ks]}
        for c in range(NCORES)
    ]
    res = run_bass_kernel_spmd(nc, in_maps, core_ids=list(range(NCORES)), trace=trace)
    slab = np.concatenate([res.results[c]["out"] for c in range(NCORES)], axis=0)
    return slab, res


def finish_host(score_slab, queue_label):
    """score_slab [K,B] (larger=nearer) -> voted labels [B] int32."""
    St = score_slab.T  # [B, K]
    k = St.shape[1]
    ncand = min(64, k)
    if ncand < k:
        part = np.argpartition(-St, ncand - 1, axis=1)[:, :ncand]
    else:
        part = np.broadcast_to(np.arange(k), St.shape).copy()
    vals = np.take_along_axis(St, part, axis=1)
    # sort candidates: descending score, ties -> lower anchor index (top_k semantics)
    order = np.lexsort((part, -vals), axis=1)[:, :KNN]
    idx = np.take_along_axis(part, order, axis=1)
    knn_labels = np.asarray(queue_label)[idx]  # [B, KNN]
    counts = np.zeros((St.shape[0], N_CLASS), dtype=np.int64)
    for c in range(N_CLASS):
        counts[:, c] = (knn_labels == c).sum(axis=1)
    return np.argmax(counts, axis=1).astype(np.int32)


def kernel(query, queue_anchor, queue_label):
    slab, _ = run_cores(query, queue_anchor, trace=False)
    return finish_host(slab, queue_label)


if __name__ == "__main__":
    # small smoke test: D small, KS small
    import time

    rng = np.random.default_rng(0)
    d_s, ks_s, b_s = 2048 + 81, 256, 512
    q = rng.uniform(1e-6, 1.0, (b_s, d_s)).astype(np.float32)
    a = rng.uniform(1e-6, 1.0, (ks_s * NCORES, d_s)).astype(np.float32)
    t0 = time.time()
    nc = _get_kernel(b_s, ks_s, d_s)
    print(f"build+compile: {time.time() - t0:.1f}s")
    slab, res = run_cores(q, a)
    # numpy oracle in fp64
    a64 = a.astype(np.float64)
    lnq = np.log(q.astype(np.float64))
    want = a64 @ lnq.T - (a64 * np.log(a64)).sum(axis=1, keepdims=True)
    err = np.abs(slab - want)
    denom = np.maximum(np.abs(want), 1e-30)
    print(f"max abs err {err.max():.4e}  max rel {np.max(err / denom):.4e}")
    print(f"sample got {slab[:2, :3]}\nwant {want[:2, :3]}")
